# revision 66
# baseline (speedup 1.0000x reference)
"""MixIT loss kernel for Trainium2 (8 NeuronCores, Bass/Tile).

Math: reference computes, for each of 16 assignment combinations k,
    mix[k,b,c,t] = sum_s A[k,c,s] * x[b,s,t]        (A tiny [16,2,4])
    loss[k] = sum_b [ snr(mix[k,b,0], m1[b]) + snr(mix[k,b,1], m2[b]) ]
    snr(y, m) = 10*log10(sum_t (y-m)^2 + 30*sum_t y^2) - 10*log10(sum_t y^2)
and returns (argmin_k, min_k).

Since mix is linear in x, every sum over T is a quadratic form in the Gram
matrix of the per-batch streams {x_0..x_3, m1, m2} over T=64000.  The device
only computes pairwise dot products; the 16-combination argmin/min
(O(16*32) flops) is finished on host.

Device layout per core (4 batches = 24 streams, interleaved host-side into
one [4, 6, T] tensor so each chunk needs only two DMA configs):
T is split as 128 partitions x 500 cols.  Cols are chunked (185, 185, 130)
so every DMA run is >= 512B (smaller runs lose DMA efficiency); the last
chunk is smallest because only its casts + matmuls trail the DMA stream.
Per chunk: DMA lands stream-major zA[128, 24, cq] f32 (both HWDGE rings),
then DVE + Act + GpSimd re-layout (and cast) to f-major bf16
zB[128, cq, 24] in parallel col ranges, and the PE runs cq/5 accumulating
bf16 matmuls with lhsT = rhs = zB[:, 5g:5g+5, :] — a contiguous [128, 120]
operand (24 streams x 5 T-cols), amortizing the fixed LDWEIGHTS cost (the
BIR verifier requires a 2D stationary operand, hence the f-major
re-layout).  bf16 runs the PE at 1 cycle/row vs fp32's 4; the
min-vs-2nd-best combo gap (1.3e-3 rel) is ~30x the bf16-induced loss
perturbation (~5e-5), so argmin is stable.  Matmuls are emitted in
data-arrival order and pinned per-chunk with scheduler-sim floor
timestamps (tile_set_cur_wait) — the list scheduler's cost model
underestimates DMA and otherwise hoists later-chunk matmuls into the
in-order PE stream, serializing the pipeline (measured 9us stall).
out[120,120] accumulates in PSUM f32 (bank A: chunks 0-1, bank B: chunk
2); entries with mismatched T-col are junk, and the host sums the 5
aligned diagonal [24,24] blocks: G[j,k] = sum_f out[24f+j, 24f+k].
The inputs are quantized to bf16 ON HOST (the device math is bf16 either
way, so results are bit-identical) and pre-tiled per chunk to the exact
[128, 24, cq] SBUF layout, which (a) halves HBM traffic and (b) makes
each partition's block one contiguous DRAM run — 2 DMA descriptors per
partition-chunk instead of 72.

Measured on HW: 49.0us (fp32 4-chunk baseline) -> ~25.1-26.3us
(median ~29.3; the spread comes from the DMA phase: all 8 cores share
HBM, so the stream shifts with cross-core skew).  Remaining floor:
~6.5us NEFF preamble + ~1.8us DMA setup + ~8.4us input DMA + ~11us
serialized casts (now the critical resource) + ~4.9us
drain/output/barrier.  5x100-col chunks pipeline casts tighter but lose
more to per-chunk overheads (measured 33.4us).
"""

import itertools
import sys

import ml_dtypes
import numpy as np

if "/opt/trn_rl_repo" not in sys.path:
    sys.path.insert(0, "/opt/trn_rl_repo")

N_CORES = 8
B = 32               # full batch
S = 4                # estimated sources
T = 64000
BL = B // N_CORES    # batches per core = 4
NJ = 6 * BL          # streams per core = 24 (per batch: 4 x, m1, m2)
P = 128
COLS = T // P        # 500
FG = 5               # T-cols fused per matmul (5*24 = 120-col operands)
# Col chunks: every (partition, stream) DMA run is cq*4 bytes; runs < 512B
# halve DMA throughput, so all chunks are >= 128 cols.  Divisible by 5 so
# matmul groups never straddle chunks.  Last chunk smallest: it is the only
# one whose transpose + matmuls sit past the end of the DMA stream.
CHUNKS = (250, 250)
NQ = len(CHUNKS)
# Pre-tiled DRAM runs are 24*cq*2 bytes, so chunks only need cq >= 11 to
# stay above the 512B DMA efficiency threshold; finer chunks pipeline the
# casts (now the critical resource) tightly behind the halved-traffic DMA.
assert sum(CHUNKS) == COLS and all(c % FG == 0 and c >= 15 for c in CHUNKS)
# Transpose engine split (fractions of each chunk's cols): measured rates
# are DVE ~2.0 ns/elem, Act ~2.2, GpSimd ~3.5; split inversely.
TSPLIT = (("vector", 0.405), ("scalar", 0.365), ("gpsimd", 0.23))
TSLICE = 28          # max cols per transpose copy instruction
SNR_MAX = 30.0

_CACHE = {}
LAST_RESULTS = None  # BassKernelResults of the most recent run (for test harness)


def _engine_cols(cq, split=None):
    """Split cq cols into per-engine contiguous ranges (multiples of FG)."""
    split = split or TSPLIT
    cuts = []
    acc = 0
    for _, frac in split[:-1]:
        acc += frac
        cuts.append(int(round(cq * acc / FG)) * FG)
    bounds = [0] + cuts + [cq]
    return [(split[i][0], bounds[i], bounds[i + 1]) for i in range(len(split))]


# Measured cast cost per col (24 streams): ns/col marginal + ns/slice
# fixed.  GpSimd gets one bigger slice per chunk (its per-instruction
# overhead is erratic); Vector/Scalar slice finely.
_CAST_NS_PER_COL = {"vector": 48.0, "scalar": 53.0, "gpsimd": 81.0}
_CAST_NS_FIXED = {"vector": 130.0, "scalar": 170.0, "gpsimd": 140.0}
_ONE_SLICE = {"gpsimd"}


# The last chunk's cast wall is the kernel's tail anchor; GpSimd is slower
# there, so shift its share to Vector/Scalar for the final chunk.
TSPLIT_LAST = (("vector", 0.435), ("scalar", 0.375), ("gpsimd", 0.19))


def _chunk_plan(cq, last=False):
    """Cast slices + matmul emission order for one chunk.

    Slices are even 5-col-multiple splits of each engine's range.  Matmul
    groups are ordered by the estimated finish time of the cast slice that
    covers them: the PE runs in-order, so emitting groups in data-arrival
    order keeps it from stalling on one late slice while later-emitted
    groups already have data.
    """
    slices = []          # (engine, s0, w)
    finish = [0.0] * cq  # per-col estimated cast finish (ns from chunk start)
    for ename, e0, e1 in _engine_cols(cq, TSPLIT_LAST if last else None):
        span = e1 - e0
        if span <= 0:
            continue
        nsl = 1 if ename in _ONE_SLICE else max(1, -(-span // TSLICE))
        base = span // nsl // FG * FG
        widths = [base] * nsl
        extra = span - base * nsl
        i = 0
        while extra > 0:
            widths[i % nsl] += FG
            extra -= FG
            i += 1

        t = 0.0
        s0 = e0
        for w in widths:
            t += w * _CAST_NS_PER_COL[ename] + _CAST_NS_FIXED[ename]
            slices.append((ename, s0, w))
            for c in range(s0, s0 + w):
                finish[c] = t
            s0 += w
    ngroups = cq // FG
    gfin = [max(finish[FG * i:FG * i + FG]) for i in range(ngroups)]
    order = sorted(range(ngroups), key=lambda i: (gfin[i], i))
    return slices, order


def _build_nc():
    from concourse import bacc, bass, tile
    import concourse.mybir as mybir

    nc = bacc.Bacc("TRN2", target_bir_lowering=False, debug=False,
                   num_devices=N_CORES)
    f32 = mybir.dt.float32
    bf16 = mybir.dt.bfloat16
    # One pre-tiled input tensor per chunk, laid out [128, 24, cq] so each
    # partition's whole 24-stream block is ONE contiguous DRAM run (~9-18KB
    # per ring half).  That collapses the input DMA from 72 descriptors per
    # partition-chunk (one per 520-740B stream run) to 2, shaving ~2us of
    # per-descriptor overhead.  The Gram over T is invariant to the
    # (chunk, partition, col) -> t mapping, so the host is free to re-tile.
    zqs = [nc.dram_tensor(f"z{q}", [P, cq, NJ], bf16, kind="ExternalInput")
           for q, cq in enumerate(CHUNKS)]
    # i-major [120, 2, 120] so the output DMA writes one contiguous 960B
    # run per partition row (480B runs would dip below the 512B threshold).
    g = nc.dram_tensor("g", [NJ * FG, 2, NJ * FG], f32, kind="ExternalOutput")

    with tile.TileContext(nc) as tc:
        with (
            tc.tile_pool(name="zb", bufs=1) as zbpool,
            tc.tile_pool(name="ps", bufs=1, space=bass.MemorySpace.PSUM) as psp,
            tc.tile_pool(name="o", bufs=1) as opool,
        ):
            acc_a = psp.tile([NJ * FG, NJ * FG], f32, tag="pa")
            acc_b = psp.tile([NJ * FG, NJ * FG], f32, tag="pb")

            # All input DMAs up front: each chunk split over both HWDGE
            # rings (sync & scalar) so descriptor generation is parallel,
            # landing DIRECTLY in the f-major bf16 matmul operand layout
            # (the host pre-tiles it), so there is NO on-device re-layout
            # pass at all.  Tiles are exactly sized so each partition half
            # is one contiguous DRAM run / descriptor.
            zbs = []
            for q, cq in enumerate(CHUNKS):
                zb = zbpool.tile([P, cq, NJ], bf16, tag=f"zb{q}")
                h = cq // 2 // FG * FG
                nc.sync.dma_start(out=zb[:, 0:h, :],
                                  in_=zqs[q].ap()[:, 0:h, :])
                nc.scalar.dma_start(out=zb[:, h:cq, :],
                                    in_=zqs[q].ap()[:, h:cq, :])
                zbs.append(zb)

            for q, cq in enumerate(CHUNKS):
                # Pin scheduler order: the list scheduler's cost model badly
                # underestimates real DMA time and will otherwise hoist a
                # later chunk's matmul ahead of earlier chunks' stragglers
                # in the in-order PE stream, serializing the whole pipeline
                # behind one long semaphore wait (measured: a 9us PE stall).
                # The floor is a scheduler-sim timestamp only; hardware
                # still runs purely on semaphores.
                tc.tile_set_cur_wait(q * 0.012)
                zb = zbs[q]
                acc = acc_b if q == NQ - 1 else acc_a
                ng = cq // FG
                for n in range(ng):
                    op = zb[:, FG * n:FG * (n + 1), :]
                    nc.tensor.matmul(
                        acc[:, :], op, op,
                        start=(n == 0 and q in (0, NQ - 1)),
                        stop=(n == ng - 1 and q in (NQ - 2, NQ - 1)),
                    )
            # Drains AFTER all cast emission so their semaphore waits (on the
            # banks' final matmuls) never block a cast engine's queue.  Bank A
            # (chunks 0-1) drains via Scalar while the PE runs chunk 2; bank B
            # via Vector at the end.  DMA cannot read PSUM, so bounce via SBUF,
            # and ship both banks with ONE output DMA (a ring config costs
            # ~0.9us and two of them serialized on the tail).
            tc.tile_set_cur_wait(NQ * 0.012)
            gout = opool.tile([NJ * FG, 2, NJ * FG], f32, tag="o")
            nc.scalar.copy(gout[:, 0, :], acc_a[:, :])
            nc.vector.tensor_copy(gout[:, 1, :], acc_b[:, :])
            nc.sync.dma_start(out=g.ap(), in_=gout[:, :, :])
    nc.compile()
    return nc


def _get_nc():
    if "nc" not in _CACHE:
        _CACHE["nc"] = _build_nc()
    return _CACHE["nc"]


def _finish_host(grams: np.ndarray):
    """grams: [N_CORES, 120, 2, 120] per-core PE blocks -> (argmin, min)."""
    grams = np.transpose(grams, (0, 2, 1, 3))
    # Collapse the fused T-col axis: G[j,k] = sum_f out[24f+j, 24f+k].
    g5 = grams.reshape(N_CORES, 2, FG, NJ, FG, NJ).astype(np.float64)
    g24 = np.einsum("cafjfk->cjk", g5)

    # Per full-batch index b: core c = b // BL, local l = b % BL.
    # Stream layout per core: x_(l,s) at 6*l+s, m1_l at 6*l+4, m2_l at 6*l+5.
    Gxx = np.empty((B, S, S), np.float64)   # sum_t x_s x_s'
    C1 = np.empty((B, S), np.float64)       # sum_t x_s m1
    C2 = np.empty((B, S), np.float64)
    M1 = np.empty((B,), np.float64)         # sum_t m1^2
    M2 = np.empty((B,), np.float64)
    for b in range(B):
        c, l = divmod(b, BL)
        gm = g24[c]
        xs = slice(6 * l, 6 * l + S)
        Gxx[b] = gm[xs, xs]
        C1[b] = gm[xs, 6 * l + 4]
        C2[b] = gm[xs, 6 * l + 5]
        M1[b] = gm[6 * l + 4, 6 * l + 4]
        M2[b] = gm[6 * l + 5, 6 * l + 5]

    combos = np.array(list(itertools.product([0, 1], repeat=S)), np.float64)
    losses = np.zeros(len(combos), np.float64)
    with np.errstate(divide="ignore"):
        for w, cc, mm in ((combos, C1, M1), (1.0 - combos, C2, M2)):
            bq = np.einsum("ks,bst,kt->kb", w, Gxx, w)        # sum_t y^2
            aq = bq - 2.0 * (w @ cc.T) + mm[None, :]          # sum_t (y-m)^2
            losses += np.sum(10.0 * np.log10(aq + SNR_MAX * bq)
                             - 10.0 * np.log10(bq), axis=1)
    k = int(np.argmin(losses))
    return np.int32(k), np.float32(losses[k])


def _ensure_trace_hook_safe():
    """If BASS_TRACE is set but this image lacks antenv.axon_hooks, install a
    null hook module so run_bass_kernel_spmd degrades to an untraced run
    instead of crashing on the import."""
    try:
        import antenv.axon_hooks  # noqa: F401
    except ImportError:
        import types

        stub = types.ModuleType("antenv.axon_hooks")
        stub.get_axon_ntff_profile_hook = lambda: None
        stub.set_axon_ntff_profile_hook = lambda h: None
        sys.modules["antenv.axon_hooks"] = stub


def kernel(estimated_sources: np.ndarray, m1: np.ndarray, m2: np.ndarray):
    global LAST_RESULTS
    _ensure_trace_hook_safe()
    from concourse.bass_utils import run_bass_kernel_spmd

    x = np.asarray(estimated_sources, dtype=np.float32)
    m1 = np.asarray(m1, dtype=np.float32)
    m2 = np.asarray(m2, dtype=np.float32)

    in_maps = []
    for c in range(N_CORES):
        sl = slice(BL * c, BL * (c + 1))
        z = np.empty((BL, 6, T), np.float32)
        z[:, 0:S] = x[sl]
        z[:, S] = m1[sl]
        z[:, S + 1] = m2[sl]
        # Pre-tile per chunk to [128, 24, cq] so each partition's block is
        # one contiguous DRAM run (t = t_q + p*cq + c; the Gram over T does
        # not care how T is partitioned), and quantize to bf16 on host —
        # the device math is bf16 either way, and this halves HBM traffic.
        z16 = z.astype(ml_dtypes.bfloat16)
        m = {}
        t0 = 0
        for q, cq in enumerate(CHUNKS):
            span = P * cq
            zq = z16[:, :, t0:t0 + span].reshape(BL, 6, P, cq)
            m[f"z{q}"] = np.ascontiguousarray(
                zq.transpose(2, 3, 0, 1).reshape(P, cq, NJ))
            t0 += span
        in_maps.append(m)

    nc = _get_nc()
    LAST_RESULTS = run_bass_kernel_spmd(nc, in_maps, list(range(N_CORES)))
    grams = np.stack([LAST_RESULTS.results[c]["g"] for c in range(N_CORES)])
    return _finish_host(grams)


# revision 67
# speedup vs baseline: 1.1648x; 1.1648x over previous
"""MixIT loss kernel for Trainium2 (8 NeuronCores, Bass/Tile).

Math: reference computes, for each of 16 assignment combinations k,
    mix[k,b,c,t] = sum_s A[k,c,s] * x[b,s,t]        (A tiny [16,2,4])
    loss[k] = sum_b [ snr(mix[k,b,0], m1[b]) + snr(mix[k,b,1], m2[b]) ]
    snr(y, m) = 10*log10(sum_t (y-m)^2 + 30*sum_t y^2) - 10*log10(sum_t y^2)
and returns (argmin_k, min_k).

Since mix is linear in x, every sum over T is a quadratic form in the Gram
matrix of the per-batch streams {x_0..x_3, m1, m2} over T=64000.  The device
only computes pairwise dot products; the 16-combination argmin/min
(O(16*32) flops) is finished on host.

Device layout per core (4 batches = 24 streams, interleaved host-side into
one [4, 6, T] tensor so each chunk needs only two DMA configs):
T is split as 128 partitions x 500 cols.  Cols are chunked (185, 185, 130)
so every DMA run is >= 512B (smaller runs lose DMA efficiency); the last
chunk is smallest because only its casts + matmuls trail the DMA stream.
Per chunk: DMA lands stream-major zA[128, 24, cq] f32 (both HWDGE rings),
then DVE + Act + GpSimd re-layout (and cast) to f-major bf16
zB[128, cq, 24] in parallel col ranges, and the PE runs cq/5 accumulating
bf16 matmuls with lhsT = rhs = zB[:, 5g:5g+5, :] — a contiguous [128, 120]
operand (24 streams x 5 T-cols), amortizing the fixed LDWEIGHTS cost (the
BIR verifier requires a 2D stationary operand, hence the f-major
re-layout).  bf16 runs the PE at 1 cycle/row vs fp32's 4; the
min-vs-2nd-best combo gap (1.3e-3 rel) is ~30x the bf16-induced loss
perturbation (~5e-5), so argmin is stable.  Matmuls are emitted in
data-arrival order and pinned per-chunk with scheduler-sim floor
timestamps (tile_set_cur_wait) — the list scheduler's cost model
underestimates DMA and otherwise hoists later-chunk matmuls into the
in-order PE stream, serializing the pipeline (measured 9us stall).
out[120,120] accumulates in PSUM f32 (bank A: chunks 0-1, bank B: chunk
2); entries with mismatched T-col are junk, and the host sums the 5
aligned diagonal [24,24] blocks: G[j,k] = sum_f out[24f+j, 24f+k].
The inputs are quantized to bf16 ON HOST (the device math is bf16 either
way, so results are bit-identical) and pre-tiled per chunk to the exact
[128, 24, cq] SBUF layout, which (a) halves HBM traffic and (b) makes
each partition's block one contiguous DRAM run — 2 DMA descriptors per
partition-chunk instead of 72.

Measured on HW: 49.0us (fp32 4-chunk baseline) -> ~25.1-26.3us
(median ~29.3; the spread comes from the DMA phase: all 8 cores share
HBM, so the stream shifts with cross-core skew).  Remaining floor:
~6.5us NEFF preamble + ~1.8us DMA setup + ~8.4us input DMA + ~11us
serialized casts (now the critical resource) + ~4.9us
drain/output/barrier.  5x100-col chunks pipeline casts tighter but lose
more to per-chunk overheads (measured 33.4us).
"""

import itertools
import sys

import ml_dtypes
import numpy as np

if "/opt/trn_rl_repo" not in sys.path:
    sys.path.insert(0, "/opt/trn_rl_repo")

N_CORES = 8
B = 32               # full batch
S = 4                # estimated sources
T = 64000
BL = B // N_CORES    # batches per core = 4
NJ = 6 * BL          # streams per core = 24 (per batch: 4 x, m1, m2)
P = 128
COLS = T // P        # 500
FG = 5               # T-cols fused per matmul (5*24 = 120-col operands)
# Col chunks: every (partition, stream) DMA run is cq*4 bytes; runs < 512B
# halve DMA throughput, so all chunks are >= 128 cols.  Divisible by 5 so
# matmul groups never straddle chunks.  Last chunk smallest: it is the only
# one whose transpose + matmuls sit past the end of the DMA stream.
CHUNKS = (280, 220)
NQ = len(CHUNKS)
# Pre-tiled DRAM runs are 24*cq*2 bytes, so chunks only need cq >= 11 to
# stay above the 512B DMA efficiency threshold; finer chunks pipeline the
# casts (now the critical resource) tightly behind the halved-traffic DMA.
assert sum(CHUNKS) == COLS and all(c % FG == 0 and c >= 15 for c in CHUNKS)
# Transpose engine split (fractions of each chunk's cols): measured rates
# are DVE ~2.0 ns/elem, Act ~2.2, GpSimd ~3.5; split inversely.
TSPLIT = (("vector", 0.405), ("scalar", 0.365), ("gpsimd", 0.23))
TSLICE = 28          # max cols per transpose copy instruction
SNR_MAX = 30.0

_CACHE = {}
LAST_RESULTS = None  # BassKernelResults of the most recent run (for test harness)


def _engine_cols(cq, split=None):
    """Split cq cols into per-engine contiguous ranges (multiples of FG)."""
    split = split or TSPLIT
    cuts = []
    acc = 0
    for _, frac in split[:-1]:
        acc += frac
        cuts.append(int(round(cq * acc / FG)) * FG)
    bounds = [0] + cuts + [cq]
    return [(split[i][0], bounds[i], bounds[i + 1]) for i in range(len(split))]


# Measured cast cost per col (24 streams): ns/col marginal + ns/slice
# fixed.  GpSimd gets one bigger slice per chunk (its per-instruction
# overhead is erratic); Vector/Scalar slice finely.
_CAST_NS_PER_COL = {"vector": 48.0, "scalar": 53.0, "gpsimd": 81.0}
_CAST_NS_FIXED = {"vector": 130.0, "scalar": 170.0, "gpsimd": 140.0}
_ONE_SLICE = {"gpsimd"}


# The last chunk's cast wall is the kernel's tail anchor; GpSimd is slower
# there, so shift its share to Vector/Scalar for the final chunk.
TSPLIT_LAST = (("vector", 0.435), ("scalar", 0.375), ("gpsimd", 0.19))


def _chunk_plan(cq, last=False):
    """Cast slices + matmul emission order for one chunk.

    Slices are even 5-col-multiple splits of each engine's range.  Matmul
    groups are ordered by the estimated finish time of the cast slice that
    covers them: the PE runs in-order, so emitting groups in data-arrival
    order keeps it from stalling on one late slice while later-emitted
    groups already have data.
    """
    slices = []          # (engine, s0, w)
    finish = [0.0] * cq  # per-col estimated cast finish (ns from chunk start)
    for ename, e0, e1 in _engine_cols(cq, TSPLIT_LAST if last else None):
        span = e1 - e0
        if span <= 0:
            continue
        nsl = 1 if ename in _ONE_SLICE else max(1, -(-span // TSLICE))
        base = span // nsl // FG * FG
        widths = [base] * nsl
        extra = span - base * nsl
        i = 0
        while extra > 0:
            widths[i % nsl] += FG
            extra -= FG
            i += 1

        t = 0.0
        s0 = e0
        for w in widths:
            t += w * _CAST_NS_PER_COL[ename] + _CAST_NS_FIXED[ename]
            slices.append((ename, s0, w))
            for c in range(s0, s0 + w):
                finish[c] = t
            s0 += w
    ngroups = cq // FG
    gfin = [max(finish[FG * i:FG * i + FG]) for i in range(ngroups)]
    order = sorted(range(ngroups), key=lambda i: (gfin[i], i))
    return slices, order


def _build_nc():
    from concourse import bacc, bass, tile
    import concourse.mybir as mybir

    nc = bacc.Bacc("TRN2", target_bir_lowering=False, debug=False,
                   num_devices=N_CORES)
    f32 = mybir.dt.float32
    bf16 = mybir.dt.bfloat16
    # One pre-tiled input tensor per chunk, laid out [128, 24, cq] so each
    # partition's whole 24-stream block is ONE contiguous DRAM run (~9-18KB
    # per ring half).  That collapses the input DMA from 72 descriptors per
    # partition-chunk (one per 520-740B stream run) to 2, shaving ~2us of
    # per-descriptor overhead.  The Gram over T is invariant to the
    # (chunk, partition, col) -> t mapping, so the host is free to re-tile.
    zqs = [nc.dram_tensor(f"z{q}", [P, cq, NJ], bf16, kind="ExternalInput")
           for q, cq in enumerate(CHUNKS)]
    # i-major [120, 2, 120] so the output DMA writes one contiguous 960B
    # run per partition row (480B runs would dip below the 512B threshold).
    g = nc.dram_tensor("g", [NJ * FG, 2, NJ * FG], f32, kind="ExternalOutput")

    with tile.TileContext(nc) as tc:
        with (
            tc.tile_pool(name="zb", bufs=1) as zbpool,
            tc.tile_pool(name="ps", bufs=1, space=bass.MemorySpace.PSUM) as psp,
            tc.tile_pool(name="o", bufs=1) as opool,
        ):
            acc_a = psp.tile([NJ * FG, NJ * FG], f32, tag="pa")
            acc_b = psp.tile([NJ * FG, NJ * FG], f32, tag="pb")

            # All input DMAs up front: each chunk split over both HWDGE
            # rings (sync & scalar) so descriptor generation is parallel,
            # landing DIRECTLY in the f-major bf16 matmul operand layout
            # (the host pre-tiles it), so there is NO on-device re-layout
            # pass at all.  Tiles are exactly sized so each partition half
            # is one contiguous DRAM run / descriptor.
            zbs = []
            for q, cq in enumerate(CHUNKS):
                zb = zbpool.tile([P, cq, NJ], bf16, tag=f"zb{q}")
                h = cq // 2 // FG * FG
                nc.sync.dma_start(out=zb[:, 0:h, :],
                                  in_=zqs[q].ap()[:, 0:h, :])
                nc.scalar.dma_start(out=zb[:, h:cq, :],
                                    in_=zqs[q].ap()[:, h:cq, :])
                zbs.append(zb)

            for q, cq in enumerate(CHUNKS):
                # Pin scheduler order: the list scheduler's cost model badly
                # underestimates real DMA time and will otherwise hoist a
                # later chunk's matmul ahead of earlier chunks' stragglers
                # in the in-order PE stream, serializing the whole pipeline
                # behind one long semaphore wait (measured: a 9us PE stall).
                # The floor is a scheduler-sim timestamp only; hardware
                # still runs purely on semaphores.
                tc.tile_set_cur_wait(q * 0.012)
                zb = zbs[q]
                acc = acc_b if q == NQ - 1 else acc_a
                ng = cq // FG
                for n in range(ng):
                    op = zb[:, FG * n:FG * (n + 1), :]
                    nc.tensor.matmul(
                        acc[:, :], op, op,
                        start=(n == 0 and q in (0, NQ - 1)),
                        stop=(n == ng - 1 and q in (NQ - 2, NQ - 1)),
                    )
            # Drains AFTER all cast emission so their semaphore waits (on the
            # banks' final matmuls) never block a cast engine's queue.  Bank A
            # (chunks 0-1) drains via Scalar while the PE runs chunk 2; bank B
            # via Vector at the end.  DMA cannot read PSUM, so bounce via SBUF,
            # and ship both banks with ONE output DMA (a ring config costs
            # ~0.9us and two of them serialized on the tail).
            tc.tile_set_cur_wait(NQ * 0.012)
            gout = opool.tile([NJ * FG, 2, NJ * FG], f32, tag="o")
            nc.scalar.copy(gout[:, 0, :], acc_a[:, :])
            nc.vector.tensor_copy(gout[:, 1, :], acc_b[:, :])
            nc.sync.dma_start(out=g.ap(), in_=gout[:, :, :])
    nc.compile()
    return nc


def _get_nc():
    if "nc" not in _CACHE:
        _CACHE["nc"] = _build_nc()
    return _CACHE["nc"]


def _finish_host(grams: np.ndarray):
    """grams: [N_CORES, 120, 2, 120] per-core PE blocks -> (argmin, min)."""
    grams = np.transpose(grams, (0, 2, 1, 3))
    # Collapse the fused T-col axis: G[j,k] = sum_f out[24f+j, 24f+k].
    g5 = grams.reshape(N_CORES, 2, FG, NJ, FG, NJ).astype(np.float64)
    g24 = np.einsum("cafjfk->cjk", g5)

    # Per full-batch index b: core c = b // BL, local l = b % BL.
    # Stream layout per core: x_(l,s) at 6*l+s, m1_l at 6*l+4, m2_l at 6*l+5.
    Gxx = np.empty((B, S, S), np.float64)   # sum_t x_s x_s'
    C1 = np.empty((B, S), np.float64)       # sum_t x_s m1
    C2 = np.empty((B, S), np.float64)
    M1 = np.empty((B,), np.float64)         # sum_t m1^2
    M2 = np.empty((B,), np.float64)
    for b in range(B):
        c, l = divmod(b, BL)
        gm = g24[c]
        xs = slice(6 * l, 6 * l + S)
        Gxx[b] = gm[xs, xs]
        C1[b] = gm[xs, 6 * l + 4]
        C2[b] = gm[xs, 6 * l + 5]
        M1[b] = gm[6 * l + 4, 6 * l + 4]
        M2[b] = gm[6 * l + 5, 6 * l + 5]

    combos = np.array(list(itertools.product([0, 1], repeat=S)), np.float64)
    losses = np.zeros(len(combos), np.float64)
    with np.errstate(divide="ignore"):
        for w, cc, mm in ((combos, C1, M1), (1.0 - combos, C2, M2)):
            bq = np.einsum("ks,bst,kt->kb", w, Gxx, w)        # sum_t y^2
            aq = bq - 2.0 * (w @ cc.T) + mm[None, :]          # sum_t (y-m)^2
            losses += np.sum(10.0 * np.log10(aq + SNR_MAX * bq)
                             - 10.0 * np.log10(bq), axis=1)
    k = int(np.argmin(losses))
    return np.int32(k), np.float32(losses[k])


def _ensure_trace_hook_safe():
    """If BASS_TRACE is set but this image lacks antenv.axon_hooks, install a
    null hook module so run_bass_kernel_spmd degrades to an untraced run
    instead of crashing on the import."""
    try:
        import antenv.axon_hooks  # noqa: F401
    except ImportError:
        import types

        stub = types.ModuleType("antenv.axon_hooks")
        stub.get_axon_ntff_profile_hook = lambda: None
        stub.set_axon_ntff_profile_hook = lambda h: None
        sys.modules["antenv.axon_hooks"] = stub


def kernel(estimated_sources: np.ndarray, m1: np.ndarray, m2: np.ndarray):
    global LAST_RESULTS
    _ensure_trace_hook_safe()
    from concourse.bass_utils import run_bass_kernel_spmd

    x = np.asarray(estimated_sources, dtype=np.float32)
    m1 = np.asarray(m1, dtype=np.float32)
    m2 = np.asarray(m2, dtype=np.float32)

    in_maps = []
    for c in range(N_CORES):
        sl = slice(BL * c, BL * (c + 1))
        z = np.empty((BL, 6, T), np.float32)
        z[:, 0:S] = x[sl]
        z[:, S] = m1[sl]
        z[:, S + 1] = m2[sl]
        # Pre-tile per chunk to [128, 24, cq] so each partition's block is
        # one contiguous DRAM run (t = t_q + p*cq + c; the Gram over T does
        # not care how T is partitioned), and quantize to bf16 on host —
        # the device math is bf16 either way, and this halves HBM traffic.
        z16 = z.astype(ml_dtypes.bfloat16)
        m = {}
        t0 = 0
        for q, cq in enumerate(CHUNKS):
            span = P * cq
            zq = z16[:, :, t0:t0 + span].reshape(BL, 6, P, cq)
            m[f"z{q}"] = np.ascontiguousarray(
                zq.transpose(2, 3, 0, 1).reshape(P, cq, NJ))
            t0 += span
        in_maps.append(m)

    nc = _get_nc()
    LAST_RESULTS = run_bass_kernel_spmd(nc, in_maps, list(range(N_CORES)))
    grams = np.stack([LAST_RESULTS.results[c]["g"] for c in range(N_CORES)])
    return _finish_host(grams)


# revision 69
# speedup vs baseline: 1.1743x; 1.0081x over previous
"""MixIT loss kernel for Trainium2 (8 NeuronCores, Bass/Tile).

Math: reference computes, for each of 16 assignment combinations k,
    mix[k,b,c,t] = sum_s A[k,c,s] * x[b,s,t]        (A tiny [16,2,4])
    loss[k] = sum_b [ snr(mix[k,b,0], m1[b]) + snr(mix[k,b,1], m2[b]) ]
    snr(y, m) = 10*log10(sum_t (y-m)^2 + 30*sum_t y^2) - 10*log10(sum_t y^2)
and returns (argmin_k, min_k).

Since mix is linear in x, every sum over T is a quadratic form in the Gram
matrix of the per-batch streams {x_0..x_3, m1, m2} over T=64000.  The device
only computes pairwise dot products; the 16-combination argmin/min
(O(16*32) flops) is finished on host.

Device layout per core (4 batches = 24 streams, interleaved host-side into
one [4, 6, T] tensor so each chunk needs only two DMA configs):
T is split as 128 partitions x 500 cols.  Cols are chunked (185, 185, 130)
so every DMA run is >= 512B (smaller runs lose DMA efficiency); the last
chunk is smallest because only its casts + matmuls trail the DMA stream.
Per chunk: DMA lands stream-major zA[128, 24, cq] f32 (both HWDGE rings),
then DVE + Act + GpSimd re-layout (and cast) to f-major bf16
zB[128, cq, 24] in parallel col ranges, and the PE runs cq/5 accumulating
bf16 matmuls with lhsT = rhs = zB[:, 5g:5g+5, :] — a contiguous [128, 120]
operand (24 streams x 5 T-cols), amortizing the fixed LDWEIGHTS cost (the
BIR verifier requires a 2D stationary operand, hence the f-major
re-layout).  bf16 runs the PE at 1 cycle/row vs fp32's 4; the
min-vs-2nd-best combo gap (1.3e-3 rel) is ~30x the bf16-induced loss
perturbation (~5e-5), so argmin is stable.  Matmuls are emitted in
data-arrival order and pinned per-chunk with scheduler-sim floor
timestamps (tile_set_cur_wait) — the list scheduler's cost model
underestimates DMA and otherwise hoists later-chunk matmuls into the
in-order PE stream, serializing the pipeline (measured 9us stall).
out[120,120] accumulates in PSUM f32 (bank A: chunks 0-1, bank B: chunk
2); entries with mismatched T-col are junk, and the host sums the 5
aligned diagonal [24,24] blocks: G[j,k] = sum_f out[24f+j, 24f+k].
The inputs are quantized to bf16 ON HOST (the device math is bf16 either
way, so results are bit-identical) and pre-tiled per chunk to the exact
[128, 24, cq] SBUF layout, which (a) halves HBM traffic and (b) makes
each partition's block one contiguous DRAM run — 2 DMA descriptors per
partition-chunk instead of 72.

Measured on HW: 49.0us (fp32 4-chunk baseline) -> ~25.1-26.3us
(median ~29.3; the spread comes from the DMA phase: all 8 cores share
HBM, so the stream shifts with cross-core skew).  Remaining floor:
~6.5us NEFF preamble + ~1.8us DMA setup + ~8.4us input DMA + ~11us
serialized casts (now the critical resource) + ~4.9us
drain/output/barrier.  5x100-col chunks pipeline casts tighter but lose
more to per-chunk overheads (measured 33.4us).
"""

import itertools
import sys

import ml_dtypes
import numpy as np

if "/opt/trn_rl_repo" not in sys.path:
    sys.path.insert(0, "/opt/trn_rl_repo")

N_CORES = 8
B = 32               # full batch
S = 4                # estimated sources
T = 64000
BL = B // N_CORES    # batches per core = 4
NJ = 6 * BL          # streams per core = 24 (per batch: 4 x, m1, m2)
P = 128
COLS = T // P        # 500
FG = 5               # T-cols fused per matmul (5*24 = 120-col operands)
# Col chunks: every (partition, stream) DMA run is cq*4 bytes; runs < 512B
# halve DMA throughput, so all chunks are >= 128 cols.  Divisible by 5 so
# matmul groups never straddle chunks.  Last chunk smallest: it is the only
# one whose transpose + matmuls sit past the end of the DMA stream.
CHUNKS = (280, 220)
NQ = len(CHUNKS)
# Pre-tiled DRAM runs are 24*cq*2 bytes, so chunks only need cq >= 11 to
# stay above the 512B DMA efficiency threshold; finer chunks pipeline the
# casts (now the critical resource) tightly behind the halved-traffic DMA.
assert sum(CHUNKS) == COLS and all(c % FG == 0 and c >= 15 for c in CHUNKS)
# Transpose engine split (fractions of each chunk's cols): measured rates
# are DVE ~2.0 ns/elem, Act ~2.2, GpSimd ~3.5; split inversely.
TSPLIT = (("vector", 0.405), ("scalar", 0.365), ("gpsimd", 0.23))
TSLICE = 28          # max cols per transpose copy instruction
SNR_MAX = 30.0

_CACHE = {}
LAST_RESULTS = None  # BassKernelResults of the most recent run (for test harness)


def _engine_cols(cq, split=None):
    """Split cq cols into per-engine contiguous ranges (multiples of FG)."""
    split = split or TSPLIT
    cuts = []
    acc = 0
    for _, frac in split[:-1]:
        acc += frac
        cuts.append(int(round(cq * acc / FG)) * FG)
    bounds = [0] + cuts + [cq]
    return [(split[i][0], bounds[i], bounds[i + 1]) for i in range(len(split))]


# Measured cast cost per col (24 streams): ns/col marginal + ns/slice
# fixed.  GpSimd gets one bigger slice per chunk (its per-instruction
# overhead is erratic); Vector/Scalar slice finely.
_CAST_NS_PER_COL = {"vector": 48.0, "scalar": 53.0, "gpsimd": 81.0}
_CAST_NS_FIXED = {"vector": 130.0, "scalar": 170.0, "gpsimd": 140.0}
_ONE_SLICE = {"gpsimd"}


# The last chunk's cast wall is the kernel's tail anchor; GpSimd is slower
# there, so shift its share to Vector/Scalar for the final chunk.
TSPLIT_LAST = (("vector", 0.435), ("scalar", 0.375), ("gpsimd", 0.19))


def _chunk_plan(cq, last=False):
    """Cast slices + matmul emission order for one chunk.

    Slices are even 5-col-multiple splits of each engine's range.  Matmul
    groups are ordered by the estimated finish time of the cast slice that
    covers them: the PE runs in-order, so emitting groups in data-arrival
    order keeps it from stalling on one late slice while later-emitted
    groups already have data.
    """
    slices = []          # (engine, s0, w)
    finish = [0.0] * cq  # per-col estimated cast finish (ns from chunk start)
    for ename, e0, e1 in _engine_cols(cq, TSPLIT_LAST if last else None):
        span = e1 - e0
        if span <= 0:
            continue
        nsl = 1 if ename in _ONE_SLICE else max(1, -(-span // TSLICE))
        base = span // nsl // FG * FG
        widths = [base] * nsl
        extra = span - base * nsl
        i = 0
        while extra > 0:
            widths[i % nsl] += FG
            extra -= FG
            i += 1

        t = 0.0
        s0 = e0
        for w in widths:
            t += w * _CAST_NS_PER_COL[ename] + _CAST_NS_FIXED[ename]
            slices.append((ename, s0, w))
            for c in range(s0, s0 + w):
                finish[c] = t
            s0 += w
    ngroups = cq // FG
    gfin = [max(finish[FG * i:FG * i + FG]) for i in range(ngroups)]
    order = sorted(range(ngroups), key=lambda i: (gfin[i], i))
    return slices, order


def _build_nc():
    from concourse import bacc, bass, tile
    import concourse.mybir as mybir

    nc = bacc.Bacc("TRN2", target_bir_lowering=False, debug=False,
                   num_devices=N_CORES)
    f32 = mybir.dt.float32
    bf16 = mybir.dt.bfloat16
    # One pre-tiled input tensor per chunk, laid out [128, 24, cq] so each
    # partition's whole 24-stream block is ONE contiguous DRAM run (~9-18KB
    # per ring half).  That collapses the input DMA from 72 descriptors per
    # partition-chunk (one per 520-740B stream run) to 2, shaving ~2us of
    # per-descriptor overhead.  The Gram over T is invariant to the
    # (chunk, partition, col) -> t mapping, so the host is free to re-tile.
    zqs = [nc.dram_tensor(f"z{q}", [P, cq, NJ], bf16, kind="ExternalInput")
           for q, cq in enumerate(CHUNKS)]
    # i-major [120, 2, 120] so the output DMA writes one contiguous 960B
    # run per partition row (480B runs would dip below the 512B threshold).
    g = nc.dram_tensor("g", [NJ * FG, 2, NJ * FG], f32, kind="ExternalOutput")

    with tile.TileContext(nc) as tc:
        with (
            tc.tile_pool(name="zb", bufs=1) as zbpool,
            tc.tile_pool(name="ps", bufs=1, space=bass.MemorySpace.PSUM) as psp,
            tc.tile_pool(name="o", bufs=1) as opool,
        ):
            acc_a = psp.tile([NJ * FG, NJ * FG], f32, tag="pa")
            acc_b = psp.tile([NJ * FG, NJ * FG], f32, tag="pb")

            # All input DMAs up front: each chunk split over both HWDGE
            # rings (sync & scalar) so descriptor generation is parallel,
            # landing DIRECTLY in the f-major bf16 matmul operand layout
            # (the host pre-tiles it), so there is NO on-device re-layout
            # pass at all.  Tiles are exactly sized so each partition half
            # is one contiguous DRAM run / descriptor.
            zbs = []
            piecing = []
            for q, cq in enumerate(CHUNKS):
                zb = zbpool.tile([P, cq, NJ], bf16, tag=f"zb{q}")
                # Each ring ships its half of the chunk as TWO sequential
                # pieces, so the first cols land in half the time and the
                # PE (the critical path) starts that much earlier.
                h = cq // 2 // FG * FG
                s1 = h // 2 // FG * FG
                r1 = h + (cq - h) // 2 // FG * FG
                nc.sync.dma_start(out=zb[:, 0:s1, :],
                                  in_=zqs[q].ap()[:, 0:s1, :])
                nc.sync.dma_start(out=zb[:, s1:h, :],
                                  in_=zqs[q].ap()[:, s1:h, :])
                nc.scalar.dma_start(out=zb[:, h:r1, :],
                                    in_=zqs[q].ap()[:, h:r1, :])
                nc.scalar.dma_start(out=zb[:, r1:cq, :],
                                    in_=zqs[q].ap()[:, r1:cq, :])
                zbs.append(zb)
                # piece col-ranges in arrival order (ring piece 1s land
                # together, then piece 2s)
                piecing.append([(0, s1), (h, r1), (s1, h), (r1, cq)])

            for q, cq in enumerate(CHUNKS):
                # Pin scheduler order: the list scheduler's cost model badly
                # underestimates real DMA time and will otherwise hoist a
                # later chunk's matmul ahead of earlier chunks' stragglers
                # in the in-order PE stream, serializing the whole pipeline
                # behind one long semaphore wait (measured: a 9us PE stall).
                # The floor is a scheduler-sim timestamp only; hardware
                # still runs purely on semaphores.
                tc.tile_set_cur_wait(q * 0.012)
                zb = zbs[q]
                acc = acc_b if q == NQ - 1 else acc_a
                order = []
                for a, b2 in piecing[q]:
                    order.extend(range(a // FG, b2 // FG))
                for n, i in enumerate(order):
                    op = zb[:, FG * i:FG * (i + 1), :]
                    nc.tensor.matmul(
                        acc[:, :], op, op,
                        start=(n == 0 and q in (0, NQ - 1)),
                        stop=(n == len(order) - 1 and q in (NQ - 2, NQ - 1)),
                    )
            # Drains AFTER all cast emission so their semaphore waits (on the
            # banks' final matmuls) never block a cast engine's queue.  Bank A
            # (chunks 0-1) drains via Scalar while the PE runs chunk 2; bank B
            # via Vector at the end.  DMA cannot read PSUM, so bounce via SBUF,
            # and ship both banks with ONE output DMA (a ring config costs
            # ~0.9us and two of them serialized on the tail).
            tc.tile_set_cur_wait(NQ * 0.012)
            gout = opool.tile([NJ * FG, 2, NJ * FG], f32, tag="o")
            nc.scalar.copy(gout[:, 0, :], acc_a[:, :])
            nc.vector.tensor_copy(gout[:, 1, :], acc_b[:, :])
            nc.sync.dma_start(out=g.ap(), in_=gout[:, :, :])
    nc.compile()
    return nc


def _get_nc():
    if "nc" not in _CACHE:
        _CACHE["nc"] = _build_nc()
    return _CACHE["nc"]


def _finish_host(grams: np.ndarray):
    """grams: [N_CORES, 120, 2, 120] per-core PE blocks -> (argmin, min)."""
    grams = np.transpose(grams, (0, 2, 1, 3))
    # Collapse the fused T-col axis: G[j,k] = sum_f out[24f+j, 24f+k].
    g5 = grams.reshape(N_CORES, 2, FG, NJ, FG, NJ).astype(np.float64)
    g24 = np.einsum("cafjfk->cjk", g5)

    # Per full-batch index b: core c = b // BL, local l = b % BL.
    # Stream layout per core: x_(l,s) at 6*l+s, m1_l at 6*l+4, m2_l at 6*l+5.
    Gxx = np.empty((B, S, S), np.float64)   # sum_t x_s x_s'
    C1 = np.empty((B, S), np.float64)       # sum_t x_s m1
    C2 = np.empty((B, S), np.float64)
    M1 = np.empty((B,), np.float64)         # sum_t m1^2
    M2 = np.empty((B,), np.float64)
    for b in range(B):
        c, l = divmod(b, BL)
        gm = g24[c]
        xs = slice(6 * l, 6 * l + S)
        Gxx[b] = gm[xs, xs]
        C1[b] = gm[xs, 6 * l + 4]
        C2[b] = gm[xs, 6 * l + 5]
        M1[b] = gm[6 * l + 4, 6 * l + 4]
        M2[b] = gm[6 * l + 5, 6 * l + 5]

    combos = np.array(list(itertools.product([0, 1], repeat=S)), np.float64)
    losses = np.zeros(len(combos), np.float64)
    with np.errstate(divide="ignore"):
        for w, cc, mm in ((combos, C1, M1), (1.0 - combos, C2, M2)):
            bq = np.einsum("ks,bst,kt->kb", w, Gxx, w)        # sum_t y^2
            aq = bq - 2.0 * (w @ cc.T) + mm[None, :]          # sum_t (y-m)^2
            losses += np.sum(10.0 * np.log10(aq + SNR_MAX * bq)
                             - 10.0 * np.log10(bq), axis=1)
    k = int(np.argmin(losses))
    return np.int32(k), np.float32(losses[k])


def _ensure_trace_hook_safe():
    """If BASS_TRACE is set but this image lacks antenv.axon_hooks, install a
    null hook module so run_bass_kernel_spmd degrades to an untraced run
    instead of crashing on the import."""
    try:
        import antenv.axon_hooks  # noqa: F401
    except ImportError:
        import types

        stub = types.ModuleType("antenv.axon_hooks")
        stub.get_axon_ntff_profile_hook = lambda: None
        stub.set_axon_ntff_profile_hook = lambda h: None
        sys.modules["antenv.axon_hooks"] = stub


def kernel(estimated_sources: np.ndarray, m1: np.ndarray, m2: np.ndarray):
    global LAST_RESULTS
    _ensure_trace_hook_safe()
    from concourse.bass_utils import run_bass_kernel_spmd

    x = np.asarray(estimated_sources, dtype=np.float32)
    m1 = np.asarray(m1, dtype=np.float32)
    m2 = np.asarray(m2, dtype=np.float32)

    in_maps = []
    for c in range(N_CORES):
        sl = slice(BL * c, BL * (c + 1))
        z = np.empty((BL, 6, T), np.float32)
        z[:, 0:S] = x[sl]
        z[:, S] = m1[sl]
        z[:, S + 1] = m2[sl]
        # Pre-tile per chunk to [128, 24, cq] so each partition's block is
        # one contiguous DRAM run (t = t_q + p*cq + c; the Gram over T does
        # not care how T is partitioned), and quantize to bf16 on host —
        # the device math is bf16 either way, and this halves HBM traffic.
        z16 = z.astype(ml_dtypes.bfloat16)
        m = {}
        t0 = 0
        for q, cq in enumerate(CHUNKS):
            span = P * cq
            zq = z16[:, :, t0:t0 + span].reshape(BL, 6, P, cq)
            m[f"z{q}"] = np.ascontiguousarray(
                zq.transpose(2, 3, 0, 1).reshape(P, cq, NJ))
            t0 += span
        in_maps.append(m)

    nc = _get_nc()
    LAST_RESULTS = run_bass_kernel_spmd(nc, in_maps, list(range(N_CORES)))
    grams = np.stack([LAST_RESULTS.results[c]["g"] for c in range(N_CORES)])
    return _finish_host(grams)


# revision 70
# speedup vs baseline: 1.1930x; 1.0160x over previous
"""MixIT loss kernel for Trainium2 (8 NeuronCores, Bass/Tile).

Math: reference computes, for each of 16 assignment combinations k,
    mix[k,b,c,t] = sum_s A[k,c,s] * x[b,s,t]        (A tiny [16,2,4])
    loss[k] = sum_b [ snr(mix[k,b,0], m1[b]) + snr(mix[k,b,1], m2[b]) ]
    snr(y, m) = 10*log10(sum_t (y-m)^2 + 30*sum_t y^2) - 10*log10(sum_t y^2)
and returns (argmin_k, min_k).

Since mix is linear in x, every sum over T is a quadratic form in the Gram
matrix of the per-batch streams {x_0..x_3, m1, m2} over T=64000.  The device
only computes pairwise dot products; the 16-combination argmin/min
(O(16*32) flops) is finished on host.

Device layout per core (4 batches = 24 streams, interleaved host-side into
one [4, 6, T] tensor so each chunk needs only two DMA configs):
T is split as 128 partitions x 500 cols.  Cols are chunked (185, 185, 130)
so every DMA run is >= 512B (smaller runs lose DMA efficiency); the last
chunk is smallest because only its casts + matmuls trail the DMA stream.
Per chunk: DMA lands stream-major zA[128, 24, cq] f32 (both HWDGE rings),
then DVE + Act + GpSimd re-layout (and cast) to f-major bf16
zB[128, cq, 24] in parallel col ranges, and the PE runs cq/5 accumulating
bf16 matmuls with lhsT = rhs = zB[:, 5g:5g+5, :] — a contiguous [128, 120]
operand (24 streams x 5 T-cols), amortizing the fixed LDWEIGHTS cost (the
BIR verifier requires a 2D stationary operand, hence the f-major
re-layout).  bf16 runs the PE at 1 cycle/row vs fp32's 4; the
min-vs-2nd-best combo gap (1.3e-3 rel) is ~30x the bf16-induced loss
perturbation (~5e-5), so argmin is stable.  Matmuls are emitted in
data-arrival order and pinned per-chunk with scheduler-sim floor
timestamps (tile_set_cur_wait) — the list scheduler's cost model
underestimates DMA and otherwise hoists later-chunk matmuls into the
in-order PE stream, serializing the pipeline (measured 9us stall).
out[120,120] accumulates in PSUM f32 (bank A: chunks 0-1, bank B: chunk
2); entries with mismatched T-col are junk, and the host sums the 5
aligned diagonal [24,24] blocks: G[j,k] = sum_f out[24f+j, 24f+k].
The inputs are quantized to bf16 ON HOST (the device math is bf16 either
way, so results are bit-identical) and pre-tiled per chunk to the exact
[128, 24, cq] SBUF layout, which (a) halves HBM traffic and (b) makes
each partition's block one contiguous DRAM run — 2 DMA descriptors per
partition-chunk instead of 72.

Measured on HW: 49.0us (fp32 4-chunk baseline) -> 23.9-25.2us.
Final structure: host pre-tiles + bf16-quantizes inputs directly into
the f-major [128, cq, 24] matmul operand layout (zero on-device
re-layout work); chunks (280, 220) sized so the PE never stalls at the
boundary (a stall resets its 1.2->2.4GHz clock ramp); each ring ships
its half-chunk as two sequential pieces so the PE starts on the first
cols ~2.3us earlier.  Remaining time is ~6.5us NEFF preamble + DMA
stream + a near-continuous PE run + ~4.6us drain/output/barrier.
"""

import itertools
import sys

import ml_dtypes
import numpy as np

if "/opt/trn_rl_repo" not in sys.path:
    sys.path.insert(0, "/opt/trn_rl_repo")

N_CORES = 8
B = 32               # full batch
S = 4                # estimated sources
T = 64000
BL = B // N_CORES    # batches per core = 4
NJ = 6 * BL          # streams per core = 24 (per batch: 4 x, m1, m2)
P = 128
COLS = T // P        # 500
FG = 5               # T-cols fused per matmul (5*24 = 120-col operands)
# Col chunks: every (partition, stream) DMA run is cq*4 bytes; runs < 512B
# halve DMA throughput, so all chunks are >= 128 cols.  Divisible by 5 so
# matmul groups never straddle chunks.  Last chunk smallest: it is the only
# one whose transpose + matmuls sit past the end of the DMA stream.
CHUNKS = (280, 220)
NQ = len(CHUNKS)
# Pre-tiled DRAM runs are 24*cq*2 bytes, so chunks only need cq >= 11 to
# stay above the 512B DMA efficiency threshold; finer chunks pipeline the
# casts (now the critical resource) tightly behind the halved-traffic DMA.
assert sum(CHUNKS) == COLS and all(c % FG == 0 and c >= 15 for c in CHUNKS)
# Transpose engine split (fractions of each chunk's cols): measured rates
# are DVE ~2.0 ns/elem, Act ~2.2, GpSimd ~3.5; split inversely.
TSPLIT = (("vector", 0.405), ("scalar", 0.365), ("gpsimd", 0.23))
TSLICE = 28          # max cols per transpose copy instruction
SNR_MAX = 30.0

_CACHE = {}
LAST_RESULTS = None  # BassKernelResults of the most recent run (for test harness)


def _engine_cols(cq, split=None):
    """Split cq cols into per-engine contiguous ranges (multiples of FG)."""
    split = split or TSPLIT
    cuts = []
    acc = 0
    for _, frac in split[:-1]:
        acc += frac
        cuts.append(int(round(cq * acc / FG)) * FG)
    bounds = [0] + cuts + [cq]
    return [(split[i][0], bounds[i], bounds[i + 1]) for i in range(len(split))]


# Measured cast cost per col (24 streams): ns/col marginal + ns/slice
# fixed.  GpSimd gets one bigger slice per chunk (its per-instruction
# overhead is erratic); Vector/Scalar slice finely.
_CAST_NS_PER_COL = {"vector": 48.0, "scalar": 53.0, "gpsimd": 81.0}
_CAST_NS_FIXED = {"vector": 130.0, "scalar": 170.0, "gpsimd": 140.0}
_ONE_SLICE = {"gpsimd"}


# The last chunk's cast wall is the kernel's tail anchor; GpSimd is slower
# there, so shift its share to Vector/Scalar for the final chunk.
TSPLIT_LAST = (("vector", 0.435), ("scalar", 0.375), ("gpsimd", 0.19))


def _chunk_plan(cq, last=False):
    """Cast slices + matmul emission order for one chunk.

    Slices are even 5-col-multiple splits of each engine's range.  Matmul
    groups are ordered by the estimated finish time of the cast slice that
    covers them: the PE runs in-order, so emitting groups in data-arrival
    order keeps it from stalling on one late slice while later-emitted
    groups already have data.
    """
    slices = []          # (engine, s0, w)
    finish = [0.0] * cq  # per-col estimated cast finish (ns from chunk start)
    for ename, e0, e1 in _engine_cols(cq, TSPLIT_LAST if last else None):
        span = e1 - e0
        if span <= 0:
            continue
        nsl = 1 if ename in _ONE_SLICE else max(1, -(-span // TSLICE))
        base = span // nsl // FG * FG
        widths = [base] * nsl
        extra = span - base * nsl
        i = 0
        while extra > 0:
            widths[i % nsl] += FG
            extra -= FG
            i += 1

        t = 0.0
        s0 = e0
        for w in widths:
            t += w * _CAST_NS_PER_COL[ename] + _CAST_NS_FIXED[ename]
            slices.append((ename, s0, w))
            for c in range(s0, s0 + w):
                finish[c] = t
            s0 += w
    ngroups = cq // FG
    gfin = [max(finish[FG * i:FG * i + FG]) for i in range(ngroups)]
    order = sorted(range(ngroups), key=lambda i: (gfin[i], i))
    return slices, order


def _build_nc():
    from concourse import bacc, bass, tile
    import concourse.mybir as mybir

    nc = bacc.Bacc("TRN2", target_bir_lowering=False, debug=False,
                   num_devices=N_CORES)
    f32 = mybir.dt.float32
    bf16 = mybir.dt.bfloat16
    # One pre-tiled input tensor per chunk, laid out [128, 24, cq] so each
    # partition's whole 24-stream block is ONE contiguous DRAM run (~9-18KB
    # per ring half).  That collapses the input DMA from 72 descriptors per
    # partition-chunk (one per 520-740B stream run) to 2, shaving ~2us of
    # per-descriptor overhead.  The Gram over T is invariant to the
    # (chunk, partition, col) -> t mapping, so the host is free to re-tile.
    zqs = [nc.dram_tensor(f"z{q}", [P, cq, NJ], bf16, kind="ExternalInput")
           for q, cq in enumerate(CHUNKS)]
    # i-major [120, 2, 120] so the output DMA writes one contiguous 960B
    # run per partition row (480B runs would dip below the 512B threshold).
    g = nc.dram_tensor("g", [NJ * FG, 2, NJ * FG], f32, kind="ExternalOutput")

    with tile.TileContext(nc) as tc:
        with (
            tc.tile_pool(name="zb", bufs=1) as zbpool,
            tc.tile_pool(name="ps", bufs=1, space=bass.MemorySpace.PSUM) as psp,
            tc.tile_pool(name="o", bufs=1) as opool,
        ):
            acc_a = psp.tile([NJ * FG, NJ * FG], f32, tag="pa")
            acc_b = psp.tile([NJ * FG, NJ * FG], f32, tag="pb")

            # All input DMAs up front: each chunk split over both HWDGE
            # rings (sync & scalar) so descriptor generation is parallel,
            # landing DIRECTLY in the f-major bf16 matmul operand layout
            # (the host pre-tiles it), so there is NO on-device re-layout
            # pass at all.  Tiles are exactly sized so each partition half
            # is one contiguous DRAM run / descriptor.
            zbs = []
            piecing = []
            for q, cq in enumerate(CHUNKS):
                zb = zbpool.tile([P, cq, NJ], bf16, tag=f"zb{q}")
                # Each ring ships its half of the chunk as TWO sequential
                # pieces, so the first cols land in half the time and the
                # PE (the critical path) starts that much earlier.
                h = cq // 2 // FG * FG
                s1 = h // 2 // FG * FG
                r1 = h + (cq - h) // 2 // FG * FG
                nc.sync.dma_start(out=zb[:, 0:s1, :],
                                  in_=zqs[q].ap()[:, 0:s1, :])
                nc.sync.dma_start(out=zb[:, s1:h, :],
                                  in_=zqs[q].ap()[:, s1:h, :])
                nc.scalar.dma_start(out=zb[:, h:r1, :],
                                    in_=zqs[q].ap()[:, h:r1, :])
                nc.scalar.dma_start(out=zb[:, r1:cq, :],
                                    in_=zqs[q].ap()[:, r1:cq, :])
                zbs.append(zb)
                # piece col-ranges in arrival order (ring piece 1s land
                # together, then piece 2s)
                piecing.append([(0, s1), (h, r1), (s1, h), (r1, cq)])

            for q, cq in enumerate(CHUNKS):
                # Pin scheduler order: the list scheduler's cost model badly
                # underestimates real DMA time and will otherwise hoist a
                # later chunk's matmul ahead of earlier chunks' stragglers
                # in the in-order PE stream, serializing the whole pipeline
                # behind one long semaphore wait (measured: a 9us PE stall).
                # The floor is a scheduler-sim timestamp only; hardware
                # still runs purely on semaphores.
                tc.tile_set_cur_wait(q * 0.012)
                zb = zbs[q]
                acc = acc_b if q == NQ - 1 else acc_a
                order = []
                for a, b2 in piecing[q]:
                    order.extend(range(a // FG, b2 // FG))
                for n, i in enumerate(order):
                    op = zb[:, FG * i:FG * (i + 1), :]
                    nc.tensor.matmul(
                        acc[:, :], op, op,
                        start=(n == 0 and q in (0, NQ - 1)),
                        stop=(n == len(order) - 1 and q in (NQ - 2, NQ - 1)),
                    )
            # Drains AFTER all cast emission so their semaphore waits (on the
            # banks' final matmuls) never block a cast engine's queue.  Bank A
            # (chunks 0-1) drains via Scalar while the PE runs chunk 2; bank B
            # via Vector at the end.  DMA cannot read PSUM, so bounce via SBUF,
            # and ship both banks with ONE output DMA (a ring config costs
            # ~0.9us and two of them serialized on the tail).
            tc.tile_set_cur_wait(NQ * 0.012)
            gout = opool.tile([NJ * FG, 2, NJ * FG], f32, tag="o")
            nc.scalar.copy(gout[:, 0, :], acc_a[:, :])
            nc.vector.tensor_copy(gout[:, 1, :], acc_b[:, :])
            nc.sync.dma_start(out=g.ap(), in_=gout[:, :, :])
    nc.compile()
    return nc


def _get_nc():
    if "nc" not in _CACHE:
        _CACHE["nc"] = _build_nc()
    return _CACHE["nc"]


def _finish_host(grams: np.ndarray):
    """grams: [N_CORES, 120, 2, 120] per-core PE blocks -> (argmin, min)."""
    grams = np.transpose(grams, (0, 2, 1, 3))
    # Collapse the fused T-col axis: G[j,k] = sum_f out[24f+j, 24f+k].
    g5 = grams.reshape(N_CORES, 2, FG, NJ, FG, NJ).astype(np.float64)
    g24 = np.einsum("cafjfk->cjk", g5)

    # Per full-batch index b: core c = b // BL, local l = b % BL.
    # Stream layout per core: x_(l,s) at 6*l+s, m1_l at 6*l+4, m2_l at 6*l+5.
    Gxx = np.empty((B, S, S), np.float64)   # sum_t x_s x_s'
    C1 = np.empty((B, S), np.float64)       # sum_t x_s m1
    C2 = np.empty((B, S), np.float64)
    M1 = np.empty((B,), np.float64)         # sum_t m1^2
    M2 = np.empty((B,), np.float64)
    for b in range(B):
        c, l = divmod(b, BL)
        gm = g24[c]
        xs = slice(6 * l, 6 * l + S)
        Gxx[b] = gm[xs, xs]
        C1[b] = gm[xs, 6 * l + 4]
        C2[b] = gm[xs, 6 * l + 5]
        M1[b] = gm[6 * l + 4, 6 * l + 4]
        M2[b] = gm[6 * l + 5, 6 * l + 5]

    combos = np.array(list(itertools.product([0, 1], repeat=S)), np.float64)
    losses = np.zeros(len(combos), np.float64)
    with np.errstate(divide="ignore"):
        for w, cc, mm in ((combos, C1, M1), (1.0 - combos, C2, M2)):
            bq = np.einsum("ks,bst,kt->kb", w, Gxx, w)        # sum_t y^2
            aq = bq - 2.0 * (w @ cc.T) + mm[None, :]          # sum_t (y-m)^2
            losses += np.sum(10.0 * np.log10(aq + SNR_MAX * bq)
                             - 10.0 * np.log10(bq), axis=1)
    k = int(np.argmin(losses))
    return np.int32(k), np.float32(losses[k])


def _ensure_trace_hook_safe():
    """If BASS_TRACE is set but this image lacks antenv.axon_hooks, install a
    null hook module so run_bass_kernel_spmd degrades to an untraced run
    instead of crashing on the import."""
    try:
        import antenv.axon_hooks  # noqa: F401
    except ImportError:
        import types

        stub = types.ModuleType("antenv.axon_hooks")
        stub.get_axon_ntff_profile_hook = lambda: None
        stub.set_axon_ntff_profile_hook = lambda h: None
        sys.modules["antenv.axon_hooks"] = stub


def kernel(estimated_sources: np.ndarray, m1: np.ndarray, m2: np.ndarray):
    global LAST_RESULTS
    _ensure_trace_hook_safe()
    from concourse.bass_utils import run_bass_kernel_spmd

    x = np.asarray(estimated_sources, dtype=np.float32)
    m1 = np.asarray(m1, dtype=np.float32)
    m2 = np.asarray(m2, dtype=np.float32)

    in_maps = []
    for c in range(N_CORES):
        sl = slice(BL * c, BL * (c + 1))
        z = np.empty((BL, 6, T), np.float32)
        z[:, 0:S] = x[sl]
        z[:, S] = m1[sl]
        z[:, S + 1] = m2[sl]
        # Pre-tile per chunk to [128, 24, cq] so each partition's block is
        # one contiguous DRAM run (t = t_q + p*cq + c; the Gram over T does
        # not care how T is partitioned), and quantize to bf16 on host —
        # the device math is bf16 either way, and this halves HBM traffic.
        z16 = z.astype(ml_dtypes.bfloat16)
        m = {}
        t0 = 0
        for q, cq in enumerate(CHUNKS):
            span = P * cq
            zq = z16[:, :, t0:t0 + span].reshape(BL, 6, P, cq)
            m[f"z{q}"] = np.ascontiguousarray(
                zq.transpose(2, 3, 0, 1).reshape(P, cq, NJ))
            t0 += span
        in_maps.append(m)

    nc = _get_nc()
    LAST_RESULTS = run_bass_kernel_spmd(nc, in_maps, list(range(N_CORES)))
    grams = np.stack([LAST_RESULTS.results[c]["g"] for c in range(N_CORES)])
    return _finish_host(grams)
